# revision 14
# baseline (speedup 1.0000x reference)
"""Low-rank attention kernel for Trainium2, distributed over 8 NeuronCores.

Math (per batch b):
    u  = q @ Wu            [N, R]
    vp = k @ Wv            [N, R]
    S  = u @ vp.T / sqrt(R)
    out = softmax(S) @ v   [N, D]

Shapes: B=4, N=4096, D=1024, R=32.

Sharding: data-parallel over batch x row-halves -> 8 shards. Core c handles
batch b = c // 2, rows [h*2048, (h+1)*2048) with h = c % 2. Each core gets its
q-shard and the full k/v for its batch. q/k are fed pre-transposed ([D, n]
layout, f16) so every matmul contraction lands on the partition axis with no
on-device transposes. The whole path runs in f16 (inputs are ~N(0,1); f16
keeps max rel err ~9e-4 on the final output, fp32r scores were 2 cyc/col).

Per-core device kernel:
  1. uT[R, 2048]  = sum_d Wu[d, :].T qT[d, :]   (K=128 d-tiles, PSUM accum)
     vpT[R, 4096] = sum_d Wv[d, :].T kT[d, :]
  2. main loop over n-chunks of 256 rows, key-PAIRS of 256 keys:
       scoresT pair [128, 2, 256] = two K=32 matmuls into one PSUM bank
       expT = Exp(scoresT / sqrt(R)) -> f16 [128, 2, 256]   (one ACTIVATE)
       out_acc[n128, d512] += expT_tile.T @ v_tile          (PSUM accum over m)
       sum_acc[n128, 2]    += expT_tile.T @ ones
     out = out_acc * (1 / sum_acc)   (softmax normalization folded at the end)
"""

import numpy as np

B, N, D, R = 4, 4096, 1024, 32
NLOC = N // 2            # rows per core
RSCALE = float(1.0 / np.sqrt(np.float32(R)))

N_CHUNK = 256            # rows of scores computed per PSUM round
D_HALF = 512             # PSUM bank width in fp32

LAST_RESULT = None       # test.py reads exec_time_ns etc. from here


def _build():
    from concourse import bacc, mybir
    from concourse.tile import TileContext

    f32 = mybir.dt.float32
    f16 = mybir.dt.float16
    EXP = mybir.ActivationFunctionType.Exp

    nc = bacc.Bacc("TRN2", target_bir_lowering=False)

    qT = nc.dram_tensor("qT", [D, NLOC], f16, kind="ExternalInput")
    kT = nc.dram_tensor("kT", [D, N], f16, kind="ExternalInput")
    v = nc.dram_tensor("v", [N, D], f16, kind="ExternalInput")
    wu = nc.dram_tensor("wu", [D, R], f16, kind="ExternalInput")
    wv = nc.dram_tensor("wv", [D, R], f16, kind="ExternalInput")
    o = nc.dram_tensor("o", [NLOC, D], f32, kind="ExternalOutput")

    DT = D // 128         # 8 d-tiles
    MQ = N // 1024        # 4 column-quarters of kT
    NCH = NLOC // N_CHUNK  # 8 main-loop chunks
    PAIRS = N // 256      # 16 key-pairs (256 keys each)
    VG = 8                # v row-groups of 512
    VPG = N // VG // 128  # 4 key-tiles per v group

    with TileContext(nc) as tc:
        with tc.tile_pool(name="singles", bufs=1) as singles, \
             tc.tile_pool(name="kqpool", bufs=4) as kqpool, \
             tc.tile_pool(name="qhpool", bufs=16) as qhpool, \
             tc.tile_pool(name="vpool", bufs=VG) as vpool, \
             tc.tile_pool(name="expp", bufs=6) as expp, \
             tc.tile_pool(name="outp", bufs=2) as outp, \
             tc.tile_pool(name="rpool", bufs=4) as rpool, \
             tc.tile_pool(name="pacc", bufs=4, space="PSUM") as pacc, \
             tc.tile_pool(name="pscore", bufs=3, space="PSUM") as pscore, \
             tc.tile_pool(name="psums", bufs=1, space="PSUM") as psums:

            # ---- constants / projection weights ----
            wu_sb = singles.tile([128, DT, R], f16, tag="wu")
            nc.sync.dma_start(out=wu_sb, in_=wu.rearrange("(t p) r -> p t r", p=128))
            wv_sb = singles.tile([128, DT, R], f16, tag="wv")
            nc.sync.dma_start(out=wv_sb, in_=wv.rearrange("(t p) r -> p t r", p=128))
            ones = singles.tile([128, 2], f16, tag="ones")
            nc.vector.memset(ones, 1.0)

            uT = singles.tile([R, NLOC], f16, tag="uT")
            vpT = singles.tile([R, N], f16, tag="vpT")

            # ---- all input DMAs issued up front, in critical-path order.
            # kT q0 gates the first projection; qT h0 gates uT chunk 0; the
            # rest streams in while the PE works through projections + chunk 0.
            # q/k go through nc.sync, v through nc.scalar: two HWDGE issue
            # FIFOs in parallel so descriptor issue (~0.7us per dma_start)
            # doesn't throttle the 380 GB/s HBM stream.
            kq = [None] * MQ   # one [128, DT, 1024] tile per kT quarter
            qt = {}            # (t, h) -> tile
            v_sb = [None] * VG

            def load_kq(qtr):
                tile = kqpool.tile([128, DT, 1024], f16, tag="kq",
                                   name=f"kq{qtr}")
                nc.sync.dma_start(
                    out=tile,
                    in_=kT[:, qtr * 1024:(qtr + 1) * 1024].rearrange(
                        "(t p) c -> p t c", p=128))
                kq[qtr] = tile

            def load_qh(h):
                for t in range(DT):
                    tile = qhpool.tile([128, 1024], f16, tag="qh",
                                       name=f"qt{h}_{t}")
                    nc.sync.dma_start(
                        out=tile, in_=qT[t * 128:(t + 1) * 128,
                                         h * 1024:(h + 1) * 1024])
                    qt[(t, h)] = tile

            def load_v(g):
                vt = vpool.tile([128, VPG, D], f16, tag="v", name=f"v{g}")
                nc.scalar.dma_start(
                    out=vt, in_=v[g * 512:(g + 1) * 512, :].rearrange(
                        "(t p) d -> p t d", p=128))
                v_sb[g] = vt

            load_kq(0)
            load_qh(0)
            load_kq(1)
            load_kq(2)
            load_kq(3)
            load_qh(1)
            for g in range(VG):
                load_v(g)

            # ---- projection helpers ----
            def u_chunk(c):
                h, off = c // 2, (c % 2) * 512
                pu = pscore.tile([128, 2, 256], f32, tag="scores", name=f"pu{c}")
                for t in range(DT):
                    nc.tensor.matmul(pu[0:R], lhsT=wu_sb[:, t, :],
                                     rhs=qt[(t, h)][:, off:off + 512],
                                     start=(t == 0), stop=(t == DT - 1))
                for s in range(2):
                    nc.vector.tensor_copy(
                        out=uT[:, c * 512 + s * 256:c * 512 + (s + 1) * 256],
                        in_=pu[0:R, s, :])

            def vp_quarter(qtr):
                for c2 in range(2):
                    pv = pscore.tile([128, 2, 256], f32, tag="scores")
                    for t in range(DT):
                        nc.tensor.matmul(pv[0:R], lhsT=wv_sb[:, t, :],
                                         rhs=kq[qtr][:, t, c2 * 512:c2 * 512 + 512],
                                         start=(t == 0), stop=(t == DT - 1))
                    off = qtr * 1024 + c2 * 512
                    for s in range(2):
                        nc.vector.tensor_copy(
                            out=vpT[:, off + s * 256:off + (s + 1) * 256],
                            in_=pv[0:R, s, :])

            # ---- phase 2: flash-style scores/softmax/AV ----
            # software-pipelined ACROSS chunks: scores/exp for pair i+2 are
            # issued before the AV matmuls of pair i (global pair index), so
            # ScalarE exp latency hides under the previous pair's AV work and
            # chunk boundaries don't drain the pipeline. hooks[(ch, pr)] lets
            # chunk 0 interleave the remaining projection quarters.
            hooks = {
                (0, 4): lambda: vp_quarter(2),
                (0, 8): lambda: vp_quarter(3),
                (0, 12): lambda: (u_chunk(2), u_chunk(3)),
            }
            all_pairs = [(ch, pr) for ch in range(NCH) for pr in range(PAIRS)]

            def scores_exp(ch, pr):
                ps = pscore.tile([128, 2, N_CHUNK], f32, tag="scores",
                                 name=f"ps{ch}_{pr}")
                for s in range(2):
                    m = 2 * pr + s
                    nc.tensor.matmul(
                        ps[:, s, :], lhsT=vpT[:, m * 128:(m + 1) * 128],
                        rhs=uT[:, ch * N_CHUNK:(ch + 1) * N_CHUNK],
                        start=True, stop=True, skip_group_check=True)
                ex = expp.tile([128, 2, N_CHUNK], f16, tag="ex",
                               name=f"ex{ch}_{pr}")
                nc.scalar.activation(out=ex, in_=ps, func=EXP, scale=RSCALE)
                return ex

            # PE issue order prologue: vp q0 first (kT q0 is the first DMA to
            # land), then u chunk 0/1 (qT h0), vp q1; vp q2/q3 and u chunk 2/3
            # interleave into chunk 0 via hooks once their data has arrived.
            vp_quarter(0)
            u_chunk(0)
            u_chunk(1)
            vp_quarter(1)

            ex_q = [scores_exp(0, 0), scores_exp(0, 1)]
            accs = sums = None
            for i, (ch, pr) in enumerate(all_pairs):
                if (ch, pr) in hooks:
                    hooks[(ch, pr)]()
                if pr == 0:
                    accs = [pacc.tile([128, D_HALF], f32, tag="acc",
                                      name=f"acc{ch}_{k}") for k in range(4)]
                    # both sums accumulators share one bank: start=True clears
                    # has_written bank-wide, so ONLY sums[0]'s first matmul has
                    # start=True; the cleared has_written makes sums[1]'s first
                    # start=False matmul overwrite rather than accumulate
                    sums_t = psums.tile([128, 4], f32, tag="sums",
                                        name=f"sum{ch}")
                    sums = [sums_t[:, 0:2], sums_t[:, 2:4]]
                ex = ex_q.pop(0)
                if i + 2 < len(all_pairs):
                    ex_q.append(scores_exp(*all_pairs[i + 2]))
                g, tg = pr // 2, (pr % 2) * 2
                for s in range(2):
                    first = (pr == 0 and s == 0)
                    last = (pr == PAIRS - 1 and s == 1)
                    for j in range(2):
                        lhs = ex[:, s, j * 128:(j + 1) * 128]
                        nc.tensor.matmul(accs[2 * j], lhsT=lhs,
                                         rhs=v_sb[g][:, tg + s, 0:D_HALF],
                                         start=first, stop=last,
                                         skip_group_check=True)
                        nc.tensor.matmul(accs[2 * j + 1], lhsT=lhs,
                                         rhs=v_sb[g][:, tg + s, D_HALF:D],
                                         start=first, stop=last,
                                         skip_group_check=True)
                        nc.tensor.matmul(sums[j], lhsT=lhs, rhs=ones,
                                         start=(first and j == 0), stop=last,
                                         skip_group_check=True)
                if pr == PAIRS - 1:
                    # normalize on DVE (keeps ScalarE free for the exps)
                    for j in range(2):
                        rc = rpool.tile([128, 1], f32, tag="rc",
                                        name=f"rc{ch}_{j}")
                        nc.vector.reciprocal(rc, sums[j][:, 0:1])
                        ob = outp.tile([128, D], f32, tag="ob",
                                       name=f"ob{ch}_{j}")
                        nc.vector.tensor_scalar_mul(ob[:, 0:D_HALF],
                                                    accs[2 * j], rc)
                        nc.vector.tensor_scalar_mul(ob[:, D_HALF:D],
                                                    accs[2 * j + 1], rc)
                        row = ch * N_CHUNK + j * 128
                        nc.sync.dma_start(out=o[row:row + 128, :], in_=ob)

    nc.finalize()
    return nc


def kernel(q, k, v, Wu, Wv):
    global LAST_RESULT
    from concourse import bass_utils

    nc = _build()

    kTs = [np.ascontiguousarray(k[b].T.astype(np.float16)) for b in range(B)]
    vs = [np.ascontiguousarray(v[b]).astype(np.float16) for b in range(B)]
    wu16 = np.ascontiguousarray(Wu.astype(np.float16))
    wv16 = np.ascontiguousarray(Wv.astype(np.float16))
    in_maps = []
    for core in range(8):
        b, h = core // 2, core % 2
        in_maps.append({
            "qT": np.ascontiguousarray(
                q[b].T[:, h * NLOC:(h + 1) * NLOC].astype(np.float16)),
            "kT": kTs[b],
            "v": vs[b],
            "wu": wu16,
            "wv": wv16,
        })

    res = bass_utils.run_bass_kernel_spmd(nc, in_maps, core_ids=list(range(8)))
    LAST_RESULT = res

    out = np.empty((B, N, D), dtype=np.float32)
    for core in range(8):
        b, h = core // 2, core % 2
        out[b, h * NLOC:(h + 1) * NLOC, :] = res.results[core]["o"]
    return out


# revision 18
# speedup vs baseline: 1.0771x; 1.0771x over previous
"""Low-rank attention kernel for Trainium2, distributed over 8 NeuronCores.

Math (per batch b):
    u  = q @ Wu            [N, R]
    vp = k @ Wv            [N, R]
    S  = u @ vp.T / sqrt(R)
    out = softmax(S) @ v   [N, D]

Shapes: B=4, N=4096, D=1024, R=32.

Sharding: data-parallel over batch x row-halves -> 8 shards. Core c handles
batch b = c // 2, rows [h*2048, (h+1)*2048) with h = c % 2. Each core gets its
q-shard and the full k/v for its batch. q/k are fed pre-transposed ([D, n]
layout, f16) so every matmul contraction lands on the partition axis with no
on-device transposes. The whole path runs in f16 (inputs are ~N(0,1); f16
keeps max rel err ~9e-4 on the final output, fp32r scores were 2 cyc/col).

Per-core device kernel:
  1. uT[R, 2048]  = sum_d Wu[d, :].T qT[d, :]   (K=128 d-tiles, PSUM accum)
     vpT[R, 4096] = sum_d Wv[d, :].T kT[d, :]
  2. main loop over n-chunks of 256 rows, key-PAIRS of 256 keys:
       scoresT pair [128, 2, 256] = two K=32 matmuls into one PSUM bank
       expT = Exp(scoresT / sqrt(R)) -> f16 [128, 2, 256]   (one ACTIVATE)
       out_acc[n128, d512] += expT_tile.T @ v_tile          (PSUM accum over m)
       sum_acc[n128, 2]    += expT_tile.T @ ones
     out = out_acc * (1 / sum_acc)   (softmax normalization folded at the end)
"""

import numpy as np

B, N, D, R = 4, 4096, 1024, 32
NLOC = N // 2            # rows per core
RSCALE = float(1.0 / np.sqrt(np.float32(R)))

N_CHUNK = 256            # rows of scores computed per PSUM round
D_HALF = 512             # PSUM bank width in fp32

LAST_RESULT = None       # test.py reads exec_time_ns etc. from here


def _build():
    from concourse import bacc, mybir
    from concourse.tile import TileContext

    f32 = mybir.dt.float32
    f16 = mybir.dt.float16
    EXP = mybir.ActivationFunctionType.Exp

    nc = bacc.Bacc("TRN2", target_bir_lowering=False)

    qT = nc.dram_tensor("qT", [D, NLOC], f16, kind="ExternalInput")
    kT = nc.dram_tensor("kT", [D, N], f16, kind="ExternalInput")
    v = nc.dram_tensor("v", [N, D], f16, kind="ExternalInput")
    wu = nc.dram_tensor("wu", [D, R], f16, kind="ExternalInput")
    wv = nc.dram_tensor("wv", [D, R], f16, kind="ExternalInput")
    o = nc.dram_tensor("o", [NLOC, D], f32, kind="ExternalOutput")

    DT = D // 128         # 8 d-tiles
    MQ = N // 1024        # 4 column-quarters of kT
    NCH = NLOC // N_CHUNK  # 8 main-loop chunks
    PAIRS = N // 256      # 16 key-pairs (256 keys each)
    VG = 8                # v row-groups of 512
    VPG = N // VG // 128  # 4 key-tiles per v group

    with TileContext(nc) as tc:
        with tc.tile_pool(name="singles", bufs=1) as singles, \
             tc.tile_pool(name="kqpool", bufs=16) as kqpool, \
             tc.tile_pool(name="qhpool", bufs=4) as qhpool, \
             tc.tile_pool(name="vpool", bufs=VG) as vpool, \
             tc.tile_pool(name="expp", bufs=6) as expp, \
             tc.tile_pool(name="outp", bufs=2) as outp, \
             tc.tile_pool(name="rpool", bufs=4) as rpool, \
             tc.tile_pool(name="pacc", bufs=4, space="PSUM") as pacc, \
             tc.tile_pool(name="pscore", bufs=3, space="PSUM") as pscore, \
             tc.tile_pool(name="psums", bufs=1, space="PSUM") as psums:

            # ---- constants / projection weights ----
            wu_sb = singles.tile([128, DT, R], f16, tag="wu")
            nc.sync.dma_start(out=wu_sb, in_=wu.rearrange("(t p) r -> p t r", p=128))
            wv_sb = singles.tile([128, DT, R], f16, tag="wv")
            nc.sync.dma_start(out=wv_sb, in_=wv.rearrange("(t p) r -> p t r", p=128))
            ones = singles.tile([128, 2], f16, tag="ones")
            nc.vector.memset(ones, 1.0)

            uT = singles.tile([R, NLOC], f16, tag="uT")
            vpT = singles.tile([R, N], f16, tag="vpT")

            # ---- all input DMAs issued up front on ONE HWDGE ring (strict
            # FIFO = strict arrival order), in critical-path order: kT q0
            # gates the first projection, qT h0 gates uT chunk 0, kT q1-3 and
            # the v groups arrive just ahead of the chunk-0 pairs that read
            # them. Tiles cover 2 d-tiles per dma_start: half the descriptor
            # issue cost of per-d-tile transfers at 2x the completion grain.
            kq = {}            # (qtr, tp) -> [128, 2, 1024] tile
            qh = {}            # (h, tp) -> [128, 2, 1024] tile
            v_sb = [None] * VG

            def load_kq(qtr):
                for tp in range(DT // 2):
                    tile = kqpool.tile([128, 2, 1024], f16, tag="kq",
                                       name=f"kq{qtr}_{tp}")
                    nc.sync.dma_start(
                        out=tile,
                        in_=kT[tp * 256:(tp + 1) * 256,
                               qtr * 1024:(qtr + 1) * 1024].rearrange(
                            "(t p) c -> p t c", p=128))
                    kq[(qtr, tp)] = tile

            def load_qh(h):
                for tp in range(DT // 2):
                    tile = qhpool.tile([128, 2, 1024], f16, tag="qh",
                                       name=f"qt{h}_{tp}")
                    nc.sync.dma_start(
                        out=tile,
                        in_=qT[tp * 256:(tp + 1) * 256,
                               h * 1024:(h + 1) * 1024].rearrange(
                            "(t p) c -> p t c", p=128))
                    qh[(h, tp)] = tile

            def load_v(g):
                vt = vpool.tile([128, VPG, D], f16, tag="v", name=f"v{g}")
                nc.sync.dma_start(
                    out=vt, in_=v[g * 512:(g + 1) * 512, :].rearrange(
                        "(t p) d -> p t d", p=128))
                v_sb[g] = vt

            load_kq(0)
            load_qh(0)
            load_kq(1)
            load_v(0)
            load_v(1)
            load_kq(2)
            load_v(2)
            load_v(3)
            load_kq(3)
            load_v(4)
            load_v(5)
            load_v(6)
            load_v(7)
            load_qh(1)

            # ---- projection helpers ----
            def u_chunk(c):
                h, off = c // 2, (c % 2) * 512
                pu = pscore.tile([128, 2, 256], f32, tag="scores", name=f"pu{c}")
                for t in range(DT):
                    nc.tensor.matmul(pu[0:R], lhsT=wu_sb[:, t, :],
                                     rhs=qh[(h, t // 2)][:, t % 2, off:off + 512],
                                     start=(t == 0), stop=(t == DT - 1))
                for s in range(2):
                    nc.vector.tensor_copy(
                        out=uT[:, c * 512 + s * 256:c * 512 + (s + 1) * 256],
                        in_=pu[0:R, s, :])

            def vp_quarter(qtr):
                for c2 in range(2):
                    pv = pscore.tile([128, 2, 256], f32, tag="scores")
                    for t in range(DT):
                        nc.tensor.matmul(pv[0:R], lhsT=wv_sb[:, t, :],
                                         rhs=kq[(qtr, t // 2)][:, t % 2,
                                                              c2 * 512:c2 * 512 + 512],
                                         start=(t == 0), stop=(t == DT - 1))
                    off = qtr * 1024 + c2 * 512
                    for s in range(2):
                        nc.vector.tensor_copy(
                            out=vpT[:, off + s * 256:off + (s + 1) * 256],
                            in_=pv[0:R, s, :])

            # ---- phase 2: flash-style scores/softmax/AV ----
            # software-pipelined ACROSS chunks: scores/exp for pair i+2 are
            # issued before the AV matmuls of pair i (global pair index), so
            # ScalarE exp latency hides under the previous pair's AV work and
            # chunk boundaries don't drain the pipeline. hooks[(ch, pr)] lets
            # chunk 0 interleave the remaining projection quarters.
            hooks = {
                (0, 4): lambda: vp_quarter(2),
                (0, 8): lambda: vp_quarter(3),
                (0, 12): lambda: (u_chunk(2), u_chunk(3)),
            }
            all_pairs = [(ch, pr) for ch in range(NCH) for pr in range(PAIRS)]

            def scores_exp(ch, pr):
                ps = pscore.tile([128, 2, N_CHUNK], f32, tag="scores",
                                 name=f"ps{ch}_{pr}")
                for s in range(2):
                    m = 2 * pr + s
                    nc.tensor.matmul(
                        ps[:, s, :], lhsT=vpT[:, m * 128:(m + 1) * 128],
                        rhs=uT[:, ch * N_CHUNK:(ch + 1) * N_CHUNK],
                        start=True, stop=True, skip_group_check=True)
                ex = expp.tile([128, 2, N_CHUNK], f16, tag="ex",
                               name=f"ex{ch}_{pr}")
                nc.scalar.activation(out=ex, in_=ps, func=EXP, scale=RSCALE)
                return ex

            # PE issue order prologue: vp q0 first (kT q0 is the first DMA to
            # land), then u chunk 0/1 (qT h0), vp q1; vp q2/q3 and u chunk 2/3
            # interleave into chunk 0 via hooks once their data has arrived.
            vp_quarter(0)
            u_chunk(0)
            u_chunk(1)
            vp_quarter(1)

            ex_q = [scores_exp(0, 0), scores_exp(0, 1)]
            accs = sums = None
            for i, (ch, pr) in enumerate(all_pairs):
                if (ch, pr) in hooks:
                    hooks[(ch, pr)]()
                if pr == 0:
                    accs = [pacc.tile([128, D_HALF], f32, tag="acc",
                                      name=f"acc{ch}_{k}") for k in range(4)]
                    # both sums accumulators share one bank: start=True clears
                    # has_written bank-wide, so ONLY sums[0]'s first matmul has
                    # start=True; the cleared has_written makes sums[1]'s first
                    # start=False matmul overwrite rather than accumulate
                    sums_t = psums.tile([128, 4], f32, tag="sums",
                                        name=f"sum{ch}")
                    sums = [sums_t[:, 0:2], sums_t[:, 2:4]]
                ex = ex_q.pop(0)
                if i + 2 < len(all_pairs):
                    ex_q.append(scores_exp(*all_pairs[i + 2]))
                g, tg = pr // 2, (pr % 2) * 2
                for s in range(2):
                    first = (pr == 0 and s == 0)
                    last = (pr == PAIRS - 1 and s == 1)
                    for j in range(2):
                        lhs = ex[:, s, j * 128:(j + 1) * 128]
                        nc.tensor.matmul(accs[2 * j], lhsT=lhs,
                                         rhs=v_sb[g][:, tg + s, 0:D_HALF],
                                         start=first, stop=last,
                                         skip_group_check=True)
                        nc.tensor.matmul(accs[2 * j + 1], lhsT=lhs,
                                         rhs=v_sb[g][:, tg + s, D_HALF:D],
                                         start=first, stop=last,
                                         skip_group_check=True)
                        nc.tensor.matmul(sums[j], lhsT=lhs, rhs=ones,
                                         start=(first and j == 0), stop=last,
                                         skip_group_check=True)
                if pr == PAIRS - 1:
                    # normalize on DVE (keeps ScalarE free for the exps)
                    for j in range(2):
                        rc = rpool.tile([128, 1], f32, tag="rc",
                                        name=f"rc{ch}_{j}")
                        nc.vector.reciprocal(rc, sums[j][:, 0:1])
                        ob = outp.tile([128, D], f32, tag="ob",
                                       name=f"ob{ch}_{j}")
                        nc.vector.tensor_scalar_mul(ob[:, 0:D_HALF],
                                                    accs[2 * j], rc)
                        nc.vector.tensor_scalar_mul(ob[:, D_HALF:D],
                                                    accs[2 * j + 1], rc)
                        row = ch * N_CHUNK + j * 128
                        nc.sync.dma_start(out=o[row:row + 128, :], in_=ob)

    nc.finalize()
    return nc


def kernel(q, k, v, Wu, Wv):
    global LAST_RESULT
    from concourse import bass_utils

    nc = _build()

    kTs = [np.ascontiguousarray(k[b].T.astype(np.float16)) for b in range(B)]
    vs = [np.ascontiguousarray(v[b]).astype(np.float16) for b in range(B)]
    wu16 = np.ascontiguousarray(Wu.astype(np.float16))
    wv16 = np.ascontiguousarray(Wv.astype(np.float16))
    in_maps = []
    for core in range(8):
        b, h = core // 2, core % 2
        in_maps.append({
            "qT": np.ascontiguousarray(
                q[b].T[:, h * NLOC:(h + 1) * NLOC].astype(np.float16)),
            "kT": kTs[b],
            "v": vs[b],
            "wu": wu16,
            "wv": wv16,
        })

    res = bass_utils.run_bass_kernel_spmd(nc, in_maps, core_ids=list(range(8)))
    LAST_RESULT = res

    out = np.empty((B, N, D), dtype=np.float32)
    for core in range(8):
        b, h = core // 2, core % 2
        out[b, h * NLOC:(h + 1) * NLOC, :] = res.results[core]["o"]
    return out


# revision 29
# speedup vs baseline: 1.1337x; 1.0525x over previous
"""Low-rank attention kernel for Trainium2, distributed over 8 NeuronCores.

Math (per batch b):
    u  = q @ Wu            [N, R]
    vp = k @ Wv            [N, R]
    S  = u @ vp.T / sqrt(R)
    out = softmax(S) @ v   [N, D]

Shapes: B=4, N=4096, D=1024, R=32.

Sharding: data-parallel over batch x row-halves -> 8 shards. Core c handles
batch b = c // 2, rows [h*2048, (h+1)*2048) with h = c % 2. Each core gets its
q-shard and the full k/v for its batch. q/k are fed pre-transposed ([D, n]
layout, f16) so every matmul contraction lands on the partition axis with no
on-device transposes. The whole path runs in f16 (inputs are ~N(0,1); f16
keeps max rel err ~9e-4 on the final output, fp32r scores were 2 cyc/col).

Per-core device kernel:
  1. uT[R, 2048]  = sum_d Wu[d, :].T qT[d, :]   (K=128 d-tiles, PSUM accum)
     vpT[R, 4096] = sum_d Wv[d, :].T kT[d, :]
  2. main loop over n-chunks of 256 rows, key-PAIRS of 256 keys:
       scoresT pair [128, 2, 256] = two K=32 matmuls into one PSUM bank
       expT = Exp(scoresT / sqrt(R)) -> f16 [128, 2, 256]   (one ACTIVATE)
       out_acc[n128, d512] += expT_tile.T @ v_tile          (PSUM accum over m)
       sum_acc[n128, 2]    += expT_tile.T @ ones
     out = out_acc * (1 / sum_acc)   (softmax normalization folded at the end)
"""

import numpy as np

B, N, D, R = 4, 4096, 1024, 32
NLOC = N // 2            # rows per core
RSCALE = float(1.0 / np.sqrt(np.float32(R)))

N_CHUNK = 256            # rows of scores computed per PSUM round
D_HALF = 512             # PSUM bank width in fp32

LAST_RESULT = None       # test.py reads exec_time_ns etc. from here


def _build():
    from concourse import bacc, mybir
    from concourse.tile import TileContext

    f32 = mybir.dt.float32
    f16 = mybir.dt.float16
    EXP = mybir.ActivationFunctionType.Exp

    nc = bacc.Bacc("TRN2", target_bir_lowering=False)

    qT = nc.dram_tensor("qT", [D, NLOC], f16, kind="ExternalInput")
    kT = nc.dram_tensor("kT", [D, N], f16, kind="ExternalInput")
    v = nc.dram_tensor("v", [N, D], f16, kind="ExternalInput")
    # Wu/Wv replicated 4x along the rank dim on the host: [D, 128]. The
    # projections then directly produce u/vp replicated over the four
    # 32-partition groups, so the scores matmul is a uniform K=128
    # contraction (no K=32 row-group switch stalling the AV LDWEIGHTS).
    # Both operands replicated makes scores 4x too big; the exp scale
    # divides that back out exactly.
    wu = nc.dram_tensor("wu", [D, 128], f16, kind="ExternalInput")
    wv = nc.dram_tensor("wv", [D, 128], f16, kind="ExternalInput")
    o = nc.dram_tensor("o", [NLOC, D], f32, kind="ExternalOutput")

    DT = D // 128         # 8 d-tiles
    MQ = N // 1024        # 4 column-quarters of kT
    NCH = NLOC // N_CHUNK  # 8 main-loop chunks
    PAIRS = N // 256      # 16 key-pairs (256 keys each)
    VG = 8                # v row-groups of 512
    VPG = N // VG // 128  # 4 key-tiles per v group

    with TileContext(nc) as tc:
        with tc.tile_pool(name="singles", bufs=1) as singles, \
             tc.tile_pool(name="kqpool", bufs=16) as kqpool, \
             tc.tile_pool(name="qhpool", bufs=4) as qhpool, \
             tc.tile_pool(name="vpool", bufs=VG) as vpool, \
             tc.tile_pool(name="expp", bufs=6) as expp, \
             tc.tile_pool(name="outp", bufs=2) as outp, \
             tc.tile_pool(name="rpool", bufs=4) as rpool, \
             tc.tile_pool(name="pacc", bufs=4, space="PSUM") as pacc, \
             tc.tile_pool(name="pscore", bufs=3, space="PSUM") as pscore, \
             tc.tile_pool(name="psums", bufs=1, space="PSUM") as psums:

            # ---- constants / projection weights ----
            wu_sb = singles.tile([128, DT, 128], f16, tag="wu")
            nc.sync.dma_start(out=wu_sb, in_=wu.rearrange("(t p) r -> p t r", p=128))
            wv_sb = singles.tile([128, DT, 128], f16, tag="wv")
            nc.sync.dma_start(out=wv_sb, in_=wv.rearrange("(t p) r -> p t r", p=128))
            ones = singles.tile([128, 2], f16, tag="ones")
            nc.vector.memset(ones, 1.0)

            # u/vp replicated over the four 32-partition groups
            uT4 = singles.tile([128, NLOC], f16, tag="uT4")
            vpT4 = singles.tile([128, N], f16, tag="vpT4")

            # ---- all input DMAs issued up front on ONE HWDGE ring (strict
            # FIFO = strict arrival order), in critical-path order: kT q0
            # gates the first projection, qT h0 gates uT chunk 0, kT q1-3 and
            # the v groups arrive just ahead of the chunk-0 pairs that read
            # them. Tiles cover 2 d-tiles per dma_start: half the descriptor
            # issue cost of per-d-tile transfers at 2x the completion grain.
            kq = {}            # (qtr, tp) -> [128, 2, 1024] tile
            qh = {}            # (h, tp) -> [128, 2, 1024] tile
            v_sb = [None] * VG

            def load_kq(qtr):
                for tp in range(DT // 2):
                    tile = kqpool.tile([128, 2, 1024], f16, tag="kq",
                                       name=f"kq{qtr}_{tp}")
                    nc.sync.dma_start(
                        out=tile,
                        in_=kT[tp * 256:(tp + 1) * 256,
                               qtr * 1024:(qtr + 1) * 1024].rearrange(
                            "(t p) c -> p t c", p=128))
                    kq[(qtr, tp)] = tile

            def load_qh(h):
                for tp in range(DT // 2):
                    tile = qhpool.tile([128, 2, 1024], f16, tag="qh",
                                       name=f"qt{h}_{tp}")
                    nc.sync.dma_start(
                        out=tile,
                        in_=qT[tp * 256:(tp + 1) * 256,
                               h * 1024:(h + 1) * 1024].rearrange(
                            "(t p) c -> p t c", p=128))
                    qh[(h, tp)] = tile

            def load_v(g):
                vt = vpool.tile([128, VPG, D], f16, tag="v", name=f"v{g}")
                nc.sync.dma_start(
                    out=vt, in_=v[g * 512:(g + 1) * 512, :].rearrange(
                        "(t p) d -> p t d", p=128))
                v_sb[g] = vt

            load_kq(0)
            load_qh(0)
            load_kq(1)
            load_v(0)
            load_v(1)
            load_kq(2)
            load_v(2)
            load_v(3)
            load_kq(3)
            load_v(4)
            load_v(5)
            load_v(6)
            load_v(7)
            load_qh(1)

            # ---- projection helpers ----
            def u_chunk(c):
                h, off = c // 2, (c % 2) * 512
                pu = pscore.tile([128, 2, 256], f32, tag="scores", name=f"pu{c}")
                for t in range(DT):
                    nc.tensor.matmul(pu, lhsT=wu_sb[:, t, :],
                                     rhs=qh[(h, t // 2)][:, t % 2, off:off + 512],
                                     start=(t == 0), stop=(t == DT - 1))
                for s in range(2):
                    nc.vector.tensor_copy(
                        out=uT4[:, c * 512 + s * 256:c * 512 + (s + 1) * 256],
                        in_=pu[:, s, :])

            def vp_quarter(qtr):
                for c2 in range(2):
                    pv = pscore.tile([128, 2, 256], f32, tag="scores")
                    for t in range(DT):
                        nc.tensor.matmul(pv, lhsT=wv_sb[:, t, :],
                                         rhs=kq[(qtr, t // 2)][:, t % 2,
                                                               c2 * 512:c2 * 512 + 512],
                                         start=(t == 0), stop=(t == DT - 1))
                    off = qtr * 1024 + c2 * 512
                    for s in range(2):
                        nc.vector.tensor_copy(
                            out=vpT4[:, off + s * 256:off + (s + 1) * 256],
                            in_=pv[:, s, :])

            # ---- phase 2: flash-style scores/softmax/AV ----
            # software-pipelined ACROSS chunks: scores/exp for pair i+2 are
            # issued before the AV matmuls of pair i (global pair index), so
            # ScalarE exp latency hides under the previous pair's AV work and
            # chunk boundaries don't drain the pipeline. hooks[(ch, pr)] lets
            # chunk 0 interleave the remaining projection quarters.
            hooks = {
                (0, 4): lambda: vp_quarter(2),
                (0, 8): lambda: vp_quarter(3),
                (0, 12): lambda: (u_chunk(2), u_chunk(3)),
            }
            all_pairs = [(ch, pr) for ch in range(NCH) for pr in range(PAIRS)]

            def scores_exp(ch, pr):
                ps = pscore.tile([128, 2, N_CHUNK], f32, tag="scores",
                                 name=f"ps{ch}_{pr}")
                for s in range(2):
                    m = 2 * pr + s
                    nc.tensor.matmul(
                        ps[:, s, :], lhsT=vpT4[:, m * 128:(m + 1) * 128],
                        rhs=uT4[:, ch * N_CHUNK:(ch + 1) * N_CHUNK],
                        start=True, stop=True, skip_group_check=True)
                ex = expp.tile([128, 2, N_CHUNK], f16, tag="ex",
                               name=f"ex{ch}_{pr}")
                # scores carry a 4x factor from the replicated projections
                nc.scalar.activation(out=ex, in_=ps, func=EXP,
                                     scale=RSCALE / 4.0)
                return ex

            # PE issue order prologue: vp q0 first (kT q0 is the first DMA to
            # land), then u chunk 0/1 (qT h0), vp q1; vp q2/q3 and u chunk 2/3
            # interleave into chunk 0 via hooks once their data has arrived.
            vp_quarter(0)
            u_chunk(0)
            u_chunk(1)
            vp_quarter(1)

            ex_q = [scores_exp(0, 0), scores_exp(0, 1)]
            accs = sums = None
            for i, (ch, pr) in enumerate(all_pairs):
                if (ch, pr) in hooks:
                    hooks[(ch, pr)]()
                if pr == 0:
                    accs = [pacc.tile([128, D_HALF], f32, tag="acc",
                                      name=f"acc{ch}_{k}") for k in range(4)]
                    # both sums accumulators share one bank: start=True clears
                    # has_written bank-wide, so ONLY sums[0]'s first matmul has
                    # start=True; the cleared has_written makes sums[1]'s first
                    # start=False matmul overwrite rather than accumulate
                    sums_t = psums.tile([128, 4], f32, tag="sums",
                                        name=f"sum{ch}")
                    sums = [sums_t[:, 0:2], sums_t[:, 2:4]]
                ex = ex_q.pop(0)
                if i + 2 < len(all_pairs):
                    ex_q.append(scores_exp(*all_pairs[i + 2]))
                g, tg = pr // 2, (pr % 2) * 2
                for s in range(2):
                    first = (pr == 0 and s == 0)
                    last = (pr == PAIRS - 1 and s == 1)
                    for j in range(2):
                        lhs = ex[:, s, j * 128:(j + 1) * 128]
                        nc.tensor.matmul(accs[2 * j], lhsT=lhs,
                                         rhs=v_sb[g][:, tg + s, 0:D_HALF],
                                         start=first, stop=last,
                                         skip_group_check=True)
                        nc.tensor.matmul(accs[2 * j + 1], lhsT=lhs,
                                         rhs=v_sb[g][:, tg + s, D_HALF:D],
                                         start=first, stop=last,
                                         skip_group_check=True)
                        nc.tensor.matmul(sums[j], lhsT=lhs, rhs=ones,
                                         start=(first and j == 0), stop=last,
                                         skip_group_check=True)
                if pr == PAIRS - 1:
                    # normalize on DVE (keeps ScalarE free for the exps)
                    for j in range(2):
                        rc = rpool.tile([128, 1], f32, tag="rc",
                                        name=f"rc{ch}_{j}")
                        nc.vector.reciprocal(rc, sums[j][:, 0:1])
                        ob = outp.tile([128, D], f32, tag="ob",
                                       name=f"ob{ch}_{j}")
                        nc.vector.tensor_scalar_mul(ob[:, 0:D_HALF],
                                                    accs[2 * j], rc)
                        nc.vector.tensor_scalar_mul(ob[:, D_HALF:D],
                                                    accs[2 * j + 1], rc)
                        row = ch * N_CHUNK + j * 128
                        nc.sync.dma_start(out=o[row:row + 128, :], in_=ob)

    nc.finalize()
    return nc


def kernel(q, k, v, Wu, Wv):
    global LAST_RESULT
    from concourse import bass_utils

    nc = _build()

    kTs = [np.ascontiguousarray(k[b].T.astype(np.float16)) for b in range(B)]
    vs = [np.ascontiguousarray(v[b]).astype(np.float16) for b in range(B)]
    wu16 = np.ascontiguousarray(np.tile(Wu.astype(np.float16), (1, 4)))
    wv16 = np.ascontiguousarray(np.tile(Wv.astype(np.float16), (1, 4)))
    in_maps = []
    for core in range(8):
        b, h = core // 2, core % 2
        in_maps.append({
            "qT": np.ascontiguousarray(
                q[b].T[:, h * NLOC:(h + 1) * NLOC].astype(np.float16)),
            "kT": kTs[b],
            "v": vs[b],
            "wu": wu16,
            "wv": wv16,
        })

    res = bass_utils.run_bass_kernel_spmd(nc, in_maps, core_ids=list(range(8)))
    LAST_RESULT = res

    out = np.empty((B, N, D), dtype=np.float32)
    for core in range(8):
        b, h = core // 2, core % 2
        out[b, h * NLOC:(h + 1) * NLOC, :] = res.results[core]["o"]
    return out
